# revision 68
# baseline (speedup 1.0000x reference)
"""Trainium2 Bass kernel for ConvBnSign (binarized 3x3 conv + sync-BN + sign).

Math: y = conv2d(x, sign(w) * alpha)  with alpha = mean|w| per out-channel,
then train-mode BatchNorm over (N,H,W), then hard_sign.

Since alpha_o > 0 is a per-channel scale, fold it into the BN affine:
  z = conv2d(x, sign(w))          (exact +-1 weights)
  out = sign(z*A + B),  A = alpha*gamma*rsqrt(alpha^2 var_z + eps),
                        B = beta - mu_z*A
and when beta == 0 (the graded inputs) further:
  out = sign(gamma) * sign(z - mu_z)   exactly (A > 0 up to sign(gamma)),
so no variance/sumsq is needed at all — only the per-channel sum.

Precision: x is split on host into three fp8(e4m3) planes
  hi = Q8(x); mid = Q8((x-hi)*2^4); lo = Q8((x-hi-mid/2^4)*2^6)
with weights sign(w)*(1, 2^-4, 2^-6) per pass, so the recombined conv input
carries ~2^-12 relative error (measured: 729 sign flips / 25.7M outputs,
rel err 1.1e-2 < 2e-2).

PE: fp8 DoubleRow matmuls contract TWO 128x128 tap-blocks per instruction at
0.5 cyc/row -> 27 tap-passes pack into 14 DoubleRow matmuls per 448-pixel
output tile (vs 18 bf16 matmuls in the bf16 hi/lo scheme); the moving AP is
4-dim [pair, row, col] so only valid pixels are streamed.  The three fp8
planes are interleaved per padded row (row stride 174 = 3*58) so pair-mates
sit at small fixed strides and the first row-range DMA unblocks the PE early.
Caveat: the accumulation group's stop matmul must not use an overlapping
(delta==1) pair AP — walrus/HW faults; PAIRS keeps a delta=174 pair last.

Tail: chunk-0 sign work is quartered and interleaved into chunk-1's conv on
the Activation engine; chunk-1 signs are split ACT/DVE in half-image pieces
(DVE computes sign(y) via the float bit trick (y & signbit) | one_bits on
the bf16 or e4m3 encoding, exact since float rounding preserves sign), with
output DMAs issued in completion order because the store stream serializes
on the single DMA device.  The final drain's excess sem waits are spread
round-robin across engines.  The last conv unit is emitted as four 2-row
PSUM groups so its drain pipelines under its own matmuls, closing the
stats->B gate for the tail sooner.

Sharding: data-parallel, 4 images per core across 8 cores; BN stats are
per-channel partial sums [128,1] fp32 all-reduced across cores (sync-BN).

TimelineSim (the graded metric): 88729 ns vs 219661 ns baseline (2.48x).
"""

import numpy as np
import ml_dtypes

import concourse.bass as bass
import concourse.mybir as mybir
import concourse.tile as tile
from concourse.vector_clock import ScopedClock
from concourse.bass_utils import run_bass_kernel_spmd

# ---- problem constants (hardcoded per contract) ----
N_CORES = 8
N_FULL = 32           # batch
CIN = 128             # input channels
COUT = 256            # output channels
H = W = 56
KH = KW = 3
BN_EPS = 1e-5

IMGS = N_FULL // N_CORES          # 4 images per core
WP = W + 2                        # 58 padded width
HP = H + 2
PADPIX = HP * WP                  # 3364
ROWSTR = 3 * WP                   # 174: planes interleaved per padded row
PLOFF = WP                        # plane p of row r at r*ROWSTR + p*PLOFF
XLEN = HP * ROWSTR + 12           # 10104 (12 tail zeros for AP margins)
PIX = H * W                       # 3136
NCHUNK = COUT // 128              # 2 chunks of 128 output channels
RTR = 8                           # output rows per tile
RT = H // RTR                     # 7 row tiles per image
NTILE = RTR * WP                  # 464 = matmul moving free dim (8 padded rows)
NVAL = RTR * W                    # 448 valid pixels per tile
NTOT = N_FULL * PIX               # 200704 elements per channel for BN stats
NPAIR = 14                        # DoubleRow matmuls per (chunk, tile)

BF16 = mybir.dt.bfloat16
F32 = mybir.dt.float32
F8 = mybir.dt.float8e4
E4M3 = ml_dtypes.float8_e4m3

SCALES = (1.0, 2.0 ** -4, 2.0 ** -6)   # hi, mid, lo weight scales

# DoubleRow pair table: ((plane_a, tap_a), (plane_b, tap_b) | None).
# Taps k=0..8 -> (dy,dx)=divmod(k,3).  Planes: 0=hi, 1=mid, 2=lo.
# The LAST pair (stop matmul) must not use the overlapping delta=1 rhs AP:
# walrus/HW faults when stop_tensor_calc pairs with an overlapping ifmap.
PAIRS = (
    [((0, k), (1, k)) for k in range(9)]      # hi/mid of same tap
    + [((2, 0), (2, 1)), ((2, 3), (2, 4)),     # lo taps, in-row neighbors
       ((2, 6), (2, 7)), ((2, 8), None),       # odd lo tap, zero-padded half
       ((2, 2), (2, 5))]                       # lo in-column pair -> stop
)


def _tap_off(plane, k, rt):
    dy, dx = divmod(k, 3)
    return (rt * RTR + dy) * ROWSTR + plane * PLOFF + dx


_MAX_DRAIN_WAITS = 1  # walrus CTRL instructions accept a single sync wait


def _split_multi_waits(nc, max_waits=1):
    """This walrus build rejects instructions with more than one sem wait.
    Hoist excess waits onto same-engine NoOps inserted immediately before the
    offending instruction (the engine blocks at the NoOp instead — identical
    ordering semantics)."""
    ctr = 0
    for bbw in nc.main_func.blocks:
        out = []
        changed = False
        for inst in bbw.instructions:
            si = inst.sync_info
            w = list(si.on_wait or []) if si else []
            if len(w) > max_waits:
                changed = True
                excess = w[: len(w) - max_waits]
                for i in range(0, len(excess), max_waits):
                    nop = mybir.InstNoOp(name=f"WFIX-{ctr}", ins=[], outs=[])
                    ctr += 1
                    nop.engine = inst.engine
                    nop.sync_info = mybir.SyncInfo(
                        on_wait=excess[i : i + max_waits], on_update=[]
                    )
                    out.append(nop)
                inst.sync_info = mybir.SyncInfo(
                    on_wait=w[len(w) - max_waits :],
                    on_update=list(si.on_update or []),
                )
            out.append(inst)
        if changed:
            bbw.instructions = out
    return ctr


class _SplitDrainTileContext(tile.TileContext):
    """TileContext whose final drain splits its sem waits across multiple
    sync-engine instructions (this walrus build caps CTRL waits at 1)."""

    def _drain_and_barrier(self, tick_clock, wait_clock):
        drain_inst = self.nc.sync.drain()
        wait_clock.add_sem_waits(
            drain_inst.ins, ScopedClock({None: tick_clock.global_clock})
        )
        si = drain_inst.ins.sync_info
        w = list(si.on_wait or [])
        if len(w) > _MAX_DRAIN_WAITS:
            drain_inst.ins.sync_info = mybir.SyncInfo(
                on_wait=w[:_MAX_DRAIN_WAITS], on_update=list(si.on_update or [])
            )
            # distribute the excess waits round-robin across engines; the
            # all_engine_barrier below joins them, so the drain semantics
            # are preserved while the wait chain runs in parallel
            engs = [self.nc.sync, self.nc.vector, self.nc.scalar,
                    self.nc.tensor, self.nc.gpsimd]
            for n, i in enumerate(range(_MAX_DRAIN_WAITS, len(w),
                                        _MAX_DRAIN_WAITS)):
                nop = engs[n % len(engs)].nop(nofuse=True)
                nop.ins.sync_info = mybir.SyncInfo(
                    on_wait=w[i : i + _MAX_DRAIN_WAITS], on_update=[]
                )
        self.nc.all_engine_barrier()
        assert self.sems is not None
        popped = self.nc._tile_sem_poison_stack.pop()
        assert popped is self._sem_poison
        self.nc.clear_and_free_semaphores(list(self.sems.allocated().values()))
        self.nc.all_engine_barrier()


def build_bass(n_cores=N_CORES, collective=True, fast_bn=True):
    """Build the per-core Bass module (SPMD: same program on every core).

    fast_bn: beta==0 specialization — sign(A*(z-mu)+0) == sign(gamma) *
    sign(z-mu) exactly (A = alpha*gamma*rsqrt(..) and alpha,rsqrt > 0), so
    the variance/sumsq pipeline is skipped entirely and only the per-channel
    sum is all-reduced.
    """
    nc = bass.Bass(num_devices=n_cores)

    xq_d = nc.dram_tensor("xq", [IMGS, CIN, XLEN], F8, kind="ExternalInput")
    ws_d = nc.dram_tensor("ws", [CIN, NCHUNK * NPAIR * 2 * 128], F8,
                          kind="ExternalInput")
    abg_d = nc.dram_tensor("abg", [128, 3 * NCHUNK], F32, kind="ExternalInput")
    out_d = nc.dram_tensor("out", [IMGS, NCHUNK, 128, PIX], F8,
                           kind="ExternalOutput")
    # tail signs offloaded to DVE (bf16 bit trick): img0 full + img1 1st half
    out2_d = nc.dram_tensor("out2", [2, 128, PIX], BF16, kind="ExternalOutput")

    with _SplitDrainTileContext(nc) as tc:
        with (
            tc.tile_pool(name="static", bufs=1) as constp,
            tc.tile_pool(name="sq", bufs=2) as sqp,
            tc.tile_pool(name="ostg", bufs=4) as op_,
            tc.tile_pool(name="pz", bufs=8, space="PSUM") as pp,
            tc.tile_pool(name="dram", bufs=1, space="DRAM") as dp,
        ):
            xp = zp = sp = constp
            # ---- constants.  Chunk-0 weights live in TWO tiles: readers
            # of a rearranged view dep on the whole tile's last write, so a
            # single split-loaded tile would stall pair 0 on the second DMA.
            w0a = constp.tile([128, 7 * 256], F8, tag="wpk0a", name="wpk0a")
            w0b = constp.tile([128, 7 * 256], F8, tag="wpk0b", name="wpk0b")
            w1 = constp.tile([128, NPAIR * 2 * 128], F8, tag="wpk1",
                             name="wpk1")
            abg_sb = constp.tile([128, 3 * NCHUNK], F32, tag="abg")
            wv = ws_d[:].rearrange("p (j r) -> p j r", j=NCHUNK)
            nc.sync.dma_start(w0a[:], wv[:, 0, : 7 * 256])

            # ---- x tiles; image 0 loads per-plane so PE starts early ----
            xt = []
            for img in range(IMGS):
                t = xp.tile([128, XLEN], F8, tag=f"x{img}", name=f"x{img}")
                xt.append(t)
            # img0 loads in contiguous row-range pieces (planes interleaved
            # per row) so unit rt=0 starts once rows 0..10 have landed
            for r0, r1 in ((0, 10), (10, 22), (22, 38), (38, HP)):
                o = r0 * ROWSTR
                e = r1 * ROWSTR if r1 < HP else XLEN
                nc.sync.dma_start(xt[0][:, o:e], xq_d[0][:, o:e])
                if r1 == 10:   # rest of chunk-0 weights after the first piece
                    nc.sync.dma_start(w0b[:], wv[:, 0, 7 * 256 :])
            nc.sync.dma_start(w1[:], wv[:, 1])
            for img in range(1, IMGS):
                nc.sync.dma_start(xt[img][:], xq_d[img])
            nc.sync.dma_start(abg_sb[:], abg_d[:])

            # ---- z buffers + stats ----
            z = [zp.tile([128, IMGS * PIX], F32, tag=f"z{j}", name=f"z{j}")
                 for j in range(NCHUNK)]
            ssum = sp.tile([128, 64], F32, tag="ssum")
            ssq = None if fast_bn else sp.tile([128, 64], F32, tag="ssq")

            # host-precomputed columns (see _prep_inputs):
            #  fast_bn: ag=sign(gamma), na2=-sign(gamma), be unused
            #  general: ag=alpha*gamma, na2=-alpha^2,     be=beta
            ag = abg_sb[:, 0:NCHUNK]
            na2 = abg_sb[:, NCHUNK : 2 * NCHUNK]
            be = abg_sb[:, 2 * NCHUNK : 3 * NCHUNK]
            inv_n = 1.0 / NTOT
            npart = IMGS * RT

            AB = {}

            def emit_sign(j, img, lo, hi, defer=None):
                """sign(z*A+B) for pixels [lo,hi) of (chunk j, img) -> DRAM."""
                A, B = AB[j]
                ostg = op_.tile([128, hi - lo], F8, tag="ostg",
                                name=f"ostg{j}_{img}_{lo}")
                nc.scalar.activation(
                    out=ostg[:], in_=z[j][:, img * PIX + lo : img * PIX + hi],
                    func=mybir.ActivationFunctionType.Sign,
                    bias=B[:, 0:1], scale=A[:, 0:1],
                )
                if defer is None:
                    nc.sync.dma_start(out_d[img, j][:, lo:hi], ostg[:])
                else:
                    defer.append((out_d[img, j][:, lo:hi], ostg))

            # chunk-0 sign work, split into quarter-images, interleaved into
            # chunk-1's conv loop so the ACT engine never bursts
            c0_pieces = [(img, lo, lo + PIX // 4)
                         for img in range(IMGS)
                         for lo in range(0, PIX, PIX // 4)]

            # Per chunk: conv -> stats AllReduce -> sign+store. Chunk 0's
            # collective + BN tail overlaps chunk 1's conv on PE.
            for j in range(NCHUNK):
                unit = 0
                for img in range(IMGS):
                    xa = xt[img][:]
                    part_dim = list(xa.ap[0])
                    for rt in range(RT):
                        # The very last unit is emitted as two half-tiles
                        # (two PSUM groups): the first half's drain overlaps
                        # the second half's matmuls, closing the stats->B
                        # gate for the tail signs sooner.  Same PE cost.
                        last_unit = (fast_bn and j == NCHUNK - 1
                                     and img == IMGS - 1 and rt == RT - 1)
                        halves = ((0, 2), (2, 4), (4, 6), (6, RTR)) \
                            if last_unit else ((0, RTR),)
                        for hi_, (r0, r1) in enumerate(halves):
                            nr = r1 - r0
                            pt = pp.tile([128, nr * W], F32, tag="pz",
                                         name=f"pz{j}_{img}_{rt}_{r0}")
                            for q, (ta, tb) in enumerate(PAIRS):
                                off_a = (_tap_off(ta[0], ta[1], rt)
                                         + r0 * ROWSTR)
                                if tb is None:
                                    # zero weights: alias half-a's data
                                    # (delta=0 adds no extra DMA dependency)
                                    delta = 0
                                else:
                                    delta = (_tap_off(tb[0], tb[1], rt)
                                             - _tap_off(ta[0], ta[1], rt))
                                # moving free = [pair, row, col]; only valid
                                # pixels are computed
                                rhs = bass.AP(
                                    xa.tensor, xa.offset + off_a,
                                    [part_dim, [delta, 2], [ROWSTR, nr],
                                     [1, W]],
                                )
                                if j == 0:
                                    wt_ = w0a if q < 7 else w0b
                                    woff = (q % 7) * 256
                                else:
                                    wt_, woff = w1, q * 256
                                lhsT = wt_[:, woff : woff + 256].rearrange(
                                    "p (two m) -> p two m", two=2
                                )
                                nc.tensor.matmul(
                                    pt[:], lhsT, rhs,
                                    start=(q == 0), stop=(q == NPAIR - 1),
                                    perf_mode=mybir.MatmulPerfMode.DoubleRow,
                                )
                            col = img * RT + rt
                            acol = 55 + hi_ if (last_unit and hi_ > 0) \
                                else j * npart + col
                            zb = img * PIX + rt * NVAL + r0 * W
                            nc.vector.tensor_scalar(
                                out=z[j][:, zb : zb + nr * W], in0=pt[:],
                                scalar1=0.0, scalar2=None,
                                op0=mybir.AluOpType.add,
                                op1=mybir.AluOpType.add,
                                accum_out=ssum[:, acol : acol + 1],
                            )
                        if not fast_bn:
                            sqt = sqp.tile([128, NVAL], F32, tag="sqt")
                            nc.scalar.activation(
                                out=sqt[:], in_=pt[:],
                                func=mybir.ActivationFunctionType.Square,
                                accum_out=ssq[:, j * npart + col
                                              : j * npart + col + 1],
                            )
                        if j == 1 and unit < len(c0_pieces):
                            emit_sign(0, *c0_pieces[unit])
                        unit += 1

                # ---- chunk-j stats: [128,SW] = (sum[, sumsq]) ----
                SW = 1 if fast_bn else 2
                fuse_b = fast_bn and not (collective and n_cores > 1)
                cc_sb = sp.tile([128, SW], F32, tag=f"ccsb{j}", name=f"ccsb{j}")
                shi = (j + 1) * npart + (
                    3 if fast_bn and j == NCHUNK - 1 else 0)
                if not fuse_b:
                    nc.vector.reduce_sum(
                        out=cc_sb[:, 0:1],
                        in_=ssum[:, j * npart : shi],
                        axis=mybir.AxisListType.X,
                    )
                if not fast_bn:
                    nc.vector.reduce_sum(
                        out=cc_sb[:, 1:2],
                        in_=ssq[:, j * npart : (j + 1) * npart],
                        axis=mybir.AxisListType.X,
                    )
                st = sp.tile([128, SW], F32, tag=f"st{j}", name=f"st{j}")
                if collective and n_cores > 1:
                    cc_in = dp.tile([128, SW], F32, tag=f"ccin{j}",
                                    name=f"ccin{j}")
                    cc_out = dp.tile([128, SW], F32, tag=f"ccout{j}",
                                     name=f"ccout{j}")
                    nc.sync.dma_start(cc_in[:], cc_sb[:])
                    nc.gpsimd.collective_compute(
                        "AllReduce", mybir.AluOpType.add,
                        replica_groups=[list(range(n_cores))],
                        ins=[cc_in.opt()], outs=[cc_out.opt()],
                    )
                    nc.sync.dma_start(st[:], cc_out[:])
                else:
                    st = cc_sb

                B = sp.tile([128, 1], F32, tag=f"B{j}", name=f"B{j}")
                if fast_bn:
                    # beta == 0: sign(A*(z-mu)) == sign(gamma)*sign(z-mu);
                    # abg carries sg=sign(gamma) and nsg=-sign(gamma).
                    # A = sg (host constant);  B = mu*nsg = -mu*sg
                    A = ag[:, j : j + 1]
                    if fuse_b:
                        # single op: B = accum(ssum * inv_n * nsg)
                        bsc = sp.tile([128, npart + 3], F32, tag=f"bsc{j}",
                                      name=f"bsc{j}")
                        nc.vector.tensor_scalar(
                            out=bsc[:, : shi - j * npart],
                            in0=ssum[:, j * npart : shi],
                            scalar1=inv_n, scalar2=na2[:, j : j + 1],
                            op0=mybir.AluOpType.mult,
                            op1=mybir.AluOpType.mult,
                            accum_out=B[:, 0:1])
                    else:
                        nc.vector.tensor_scalar(
                            out=B[:], in0=st[:, 0:1], scalar1=inv_n,
                            scalar2=na2[:, j : j + 1],
                            op0=mybir.AluOpType.mult, op1=mybir.AluOpType.mult)
                else:
                    A = sp.tile([128, 1], F32, tag=f"A{j}", name=f"A{j}")
                    # ms=(mu,m2); nv=mu^2-m2=-var; v2=nv*(-a2)+eps;
                    # A = ag/sqrt(v2); B = beta - mu*A
                    ms = sp.tile([128, 2], F32, tag=f"ms{j}", name=f"ms{j}")
                    nv = sp.tile([128, 1], F32, tag=f"nv{j}", name=f"nv{j}")
                    tmp = sp.tile([128, 1], F32, tag=f"tmp{j}", name=f"tmp{j}")
                    nc.vector.tensor_scalar(out=ms[:], in0=st[:],
                                            scalar1=inv_n, scalar2=None,
                                            op0=mybir.AluOpType.mult)
                    nc.vector.scalar_tensor_tensor(
                        out=nv[:], in0=ms[:, 0:1], scalar=ms[:, 0:1],
                        in1=ms[:, 1:2], op0=mybir.AluOpType.mult,
                        op1=mybir.AluOpType.subtract)
                    nc.vector.tensor_scalar(
                        out=tmp[:], in0=nv[:], scalar1=na2[:, j : j + 1],
                        scalar2=float(BN_EPS), op0=mybir.AluOpType.mult,
                        op1=mybir.AluOpType.add)
                    nc.scalar.sqrt(tmp[:], tmp[:])
                    nc.vector.reciprocal(tmp[:], tmp[:])  # rsqrt(a^2 var+eps)
                    nc.vector.tensor_scalar(out=A[:], in0=tmp[:],
                                            scalar1=ag[:, j : j + 1],
                                            scalar2=None,
                                            op0=mybir.AluOpType.mult)
                    nc.vector.tensor_tensor(out=tmp[:], in0=ms[:, 0:1],
                                            in1=A[:], op=mybir.AluOpType.mult)
                    nc.vector.tensor_tensor(out=B[:], in0=be[:, j : j + 1],
                                            in1=tmp[:],
                                            op=mybir.AluOpType.subtract)
                AB[j] = (A, B)

                # chunk-1 signs run in the tail, split ACT/DVE.  DVE bit
                # trick: sign(y) == (y & 0x8000) | 0x3f80 on the bf16
                # encoding (rounding y to bf16 preserves its sign exactly).
                if j == NCHUNK - 1:
                    def dve_sign(row, img, lo, hi, f8=False):
                        n = hi - lo
                        ydt, odt = (F8, mybir.dt.int8) if f8 else \
                                   (BF16, mybir.dt.int16)
                        sbit, obit = (0x80, 0x38) if f8 else (0x8000, 0x3F80)
                        ybf = sqp.tile([128, n], ydt, tag="ybf",
                                       name=f"ybf{row}_{lo}")
                        nc.vector.tensor_scalar(
                            out=ybf[:],
                            in0=z[j][:, img * PIX + lo : img * PIX + hi],
                            scalar1=A[:, 0:1], scalar2=B[:, 0:1],
                            op0=mybir.AluOpType.mult, op1=mybir.AluOpType.add,
                        )
                        osg2 = op_.tile([128, n], ydt, tag="osg2",
                                        name=f"osg2_{row}_{lo}")
                        nc.vector.tensor_scalar(
                            out=osg2[:].bitcast(odt),
                            in0=ybf[:].bitcast(odt),
                            scalar1=sbit, scalar2=obit,
                            op0=mybir.AluOpType.bitwise_and,
                            op1=mybir.AluOpType.bitwise_or,
                        )
                        dst = (out_d[img, j][:, lo:hi] if f8
                               else out2_d[row][:, lo:hi])
                        dq.append((dst, osg2))

                    # Half-image pieces, imgs 0-1 on DVE (fp32 tensor ops get
                    # the 2x SBUF mode there) and imgs 2-3 on ACT; output
                    # DMAs issued interleaved in expected completion order so
                    # the (serialized, bandwidth-bound) store stream starts
                    # as early as possible.
                    dq, aq = [], []
                    H2 = PIX // 2
                    for lo in (0, H2):
                        dve_sign(0, 0, lo, lo + H2)           # img0 bf16
                    for lo in (0, H2):
                        dve_sign(1, 1, lo, lo + H2, f8=True)  # img1 fp8
                    for img, lo in ((2, 0), (2, H2), (3, 0), (3, H2)):
                        emit_sign(j, img, lo, lo + H2, defer=aq)
                    for a, d in zip(aq, dq):
                        nc.sync.dma_start(*a)
                        nc.sync.dma_start(*d)

    _split_multi_waits(nc)
    return nc


def _prep_inputs(x, weight, gamma, beta, fast_bn=True):
    """Host-side prep: alpha/sign folding, padding, fp8 hi/mid/lo split."""
    x = np.ascontiguousarray(x, dtype=np.float32)
    weight = np.ascontiguousarray(weight, dtype=np.float32)

    alpha = np.abs(weight).mean(axis=(1, 2, 3)).astype(np.float32)      # [256]
    sgn = np.where(weight >= 0, np.float32(1), np.float32(-1))          # [256,128,3,3]

    # DoubleRow-packed weights: wpk[cin, chunk, pair, half, m] = sgn * scale
    wpk = np.zeros((CIN, NCHUNK, NPAIR, 2, 128), np.float32)
    for j in range(NCHUNK):
        for q, (ta, tb) in enumerate(PAIRS):
            for h, t in ((0, ta), (1, tb)):
                if t is None:
                    continue
                plane, k = t
                dy, dx = divmod(k, 3)
                wpk[:, j, q, h, :] = (
                    sgn[j * 128 : (j + 1) * 128, :, dy, dx].T * SCALES[plane]
                )
    ws = np.ascontiguousarray(
        wpk.reshape(CIN, NCHUNK * NPAIR * 2 * 128)
    ).astype(E4M3)

    # abg[p, j] columns (channel o = j*128+p):
    #  fast_bn: [sign(gamma) | -sign(gamma) | 0]
    #  general: [alpha*gamma | -alpha^2    | beta]
    def chunked(v):
        return np.ascontiguousarray(v.reshape(NCHUNK, 128).T)  # [128, 2]
    gamma = np.asarray(gamma, np.float32)
    if fast_bn:
        sg = np.where(gamma >= 0, np.float32(1), np.float32(-1))
        cols = [chunked(sg), chunked(-sg), chunked(np.zeros_like(sg))]
    else:
        cols = [chunked(alpha * gamma), chunked(-alpha * alpha),
                chunked(np.asarray(beta, np.float32))]
    abg = np.concatenate(cols, axis=1).astype(np.float32)               # [128, 6]

    # fp8 hi/mid/lo split of the padded input, planes interleaved per row
    xpad = np.zeros((N_FULL, CIN, HP * WP), np.float32)
    xpad.reshape(N_FULL, CIN, HP, WP)[:, :, 1 : H + 1, 1 : W + 1] = x
    hi_q = xpad.astype(E4M3)
    r1 = xpad - hi_q.astype(np.float32)
    mid_q = (r1 * 16.0).astype(E4M3)
    r2 = r1 - mid_q.astype(np.float32) * (1.0 / 16.0)
    lo_q = (r2 * 64.0).astype(E4M3)

    xq = np.zeros((N_FULL, CIN, XLEN), E4M3)
    xv = xq[:, :, : HP * ROWSTR].reshape(N_FULL, CIN, HP, 3, WP)
    xv[:, :, :, 0, :] = hi_q.reshape(N_FULL, CIN, HP, WP)
    xv[:, :, :, 1, :] = mid_q.reshape(N_FULL, CIN, HP, WP)
    xv[:, :, :, 2, :] = lo_q.reshape(N_FULL, CIN, HP, WP)

    in_maps = []
    for c in range(N_CORES):
        sl = slice(c * IMGS, (c + 1) * IMGS)
        in_maps.append({
            "xq": np.ascontiguousarray(xq[sl]),
            "ws": ws,
            "abg": abg,
        })
    return in_maps


def kernel(x, weight, gamma, beta):
    fast_bn = bool(np.all(np.asarray(beta) == 0))
    in_maps = _prep_inputs(x, weight, gamma, beta, fast_bn=fast_bn)
    nc = build_bass(fast_bn=fast_bn)
    res = run_bass_kernel_spmd(nc, in_maps, core_ids=list(range(N_CORES)))
    out = np.empty((N_FULL, COUT, H, W), np.float32)
    for c in range(N_CORES):
        o = res.results[c]["out"].astype(np.float32)  # [IMGS,2,128,3136] +-1
        o2 = res.results[c]["out2"].astype(np.float32)  # DVE-signed img0
        o[0, 1] = o2[0]
        out[c * IMGS : (c + 1) * IMGS] = o.reshape(IMGS, COUT, H, W)
    return out
